# revision 1
# baseline (speedup 1.0000x reference)
"""Trainium2 Bass kernel for DifferentialEntropyRegularization (retrieval_knn).

Problem: x [16384, 512] f32.
  dots = x @ x.T, diagonal masked, I = argmax(dots, axis=1)
  rho = ||x - x[I] + 1e-6||_2;  out = 0.1 * (-mean(log(rho + 1e-8)))

Strategy (8 NeuronCores, SPMD):
  - Shard rows: core c owns rows [c*2048, (c+1)*2048).
  - Host pre-transposes x into K-chunked bf16 layout; each core matmuls its
    2048-row slab against all 16384 columns (bf16 inputs, fp32 PSUM accum).
  - Per 128-row group, the full dots row-block [128, 16384] is materialized
    in SBUF as bf16 (ScalarE copies from PSUM).
  - DVE max8 gives top-8 per row: top-1 is always the self-dot (||x||^2 ~ 512
    >> max cross-dot ~ 130), so top-2 is the masked argmax value. max_index
    returns its first-occurrence column = the neighbor index.
  - GPSIMD indirect DMA gathers x[I] rows (fp32) from DRAM; fp32 distance,
    ScalarE Ln; per-row log(rho^2) is written out.
  - Host: loss = -0.1 * 0.5 * mean(log(rho^2)).
    (log(rho + 1e-8) == 0.5*log(rho^2) to < fp32 ulp since rho ~ 28 >> 1e-8.)
"""

from contextlib import ExitStack

import numpy as np
import ml_dtypes

import concourse.bass as bass
import concourse.mybir as mybir
import concourse.tile as tile
from concourse.bass_utils import run_bass_kernel_spmd

P = 128


def build_program(N=16384, D=512, R=2048, JW=1024, SUPER=4):
    """One SPMD program; per-core inputs differ in data only.

    N: total rows; D: features; R: rows per core; JW: column chunk width
    (multiple of 512); SUPER: row-groups per rhs streaming pass.
    """
    KC = D // P              # K chunks of 128
    NG = R // P              # 128-row groups per core
    NJ = N // JW             # column chunks
    NS = NG // SUPER         # rhs streaming passes
    assert D % P == 0 and R % P == 0 and N % JW == 0 and JW % 512 == 0
    assert NG % SUPER == 0

    f32 = mybir.dt.float32
    bf16 = mybir.dt.bfloat16
    Alu = mybir.AluOpType

    nc = bass.Bass()
    lhsT = nc.declare_dram_parameter("lhsT", [KC, P, R], bf16, isOutput=False)
    rhsT = nc.declare_dram_parameter("rhsT", [KC, P, N], bf16, isOutput=False)
    xrows = nc.declare_dram_parameter("xrows", [R, D], f32, isOutput=False)
    xfull = nc.declare_dram_parameter("xfull", [N, D], f32, isOutput=False)
    lnr = nc.declare_dram_parameter("lnr", [R], f32, isOutput=True)

    with tile.TileContext(nc) as tc, ExitStack() as ctx:
        const_pool = ctx.enter_context(tc.tile_pool(name="const", bufs=1))
        rhs_pool = ctx.enter_context(tc.tile_pool(name="rhs", bufs=3))
        row_pool = ctx.enter_context(tc.tile_pool(name="row", bufs=SUPER))
        psum_pool = ctx.enter_context(tc.tile_pool(name="ps", bufs=3, space="PSUM"))
        scr_pool = ctx.enter_context(tc.tile_pool(name="scr", bufs=1, space="PSUM"))
        ext_pool = ctx.enter_context(tc.tile_pool(name="ext", bufs=2))

        # Resident lhsT: [P, KC, R] bf16 (single DMA: fewer sem waits downstream)
        lhsT_sb = const_pool.tile([P, KC, R], bf16)
        nc.sync.dma_start(
            out=lhsT_sb[:, :, :], in_=lhsT[:, :, :].rearrange("kc p r -> p kc r")
        )

        def extract(g, rb):
            """Argmax -> gather -> log-distance for row group g."""
            m8 = ext_pool.tile([P, 8], bf16, tag="m8")
            nc.vector.max(m8[:], rb[:])
            idx8 = ext_pool.tile([P, 8], mybir.dt.uint32, tag="idx8")
            nc.vector.max_index(idx8[:], m8[:], rb[:])
            # top-2 = cross-max; its first-occurrence column is the neighbor
            idx32 = ext_pool.tile([P, 1], mybir.dt.int32, tag="idx32")
            nc.vector.tensor_copy(idx32[:], idx8[:, 1:2])

            gath = ext_pool.tile([P, D], f32, tag="gath")
            nc.gpsimd.indirect_dma_start(
                out=gath[:],
                out_offset=None,
                in_=xfull[:, :],
                in_offset=bass.IndirectOffsetOnAxis(ap=idx32[:, :1], axis=0),
            )
            own = ext_pool.tile([P, D], f32, tag="own")
            nc.sync.dma_start(out=own[:], in_=xrows[g * P:(g + 1) * P, :])

            # diff = (own + 1e-6) - gath
            diff = ext_pool.tile([P, D], f32, tag="diff")
            nc.vector.scalar_tensor_tensor(
                diff[:], own[:], 1e-6, gath[:], op0=Alu.add, op1=Alu.subtract
            )
            # rho2 = sum(diff * diff)
            sq = ext_pool.tile([P, D], f32, tag="sq")
            rho2 = ext_pool.tile([P, 1], f32, tag="rho2")
            nc.vector.scalar_tensor_tensor(
                sq[:], diff[:], 1.0, diff[:], op0=Alu.mult, op1=Alu.mult,
                accum_out=rho2[:],
            )
            lnt = ext_pool.tile([P, 1], f32, tag="lnt")
            nc.scalar.activation(lnt[:], rho2[:], mybir.ActivationFunctionType.Ln)
            nc.sync.dma_start(out=lnr[g * P:(g + 1) * P, None], in_=lnt[:, :1])

        for s in range(NS):
            rowbufs = [
                row_pool.tile([P, N], bf16, name=f"rowbuf_{s}_{gi}", tag="rowbuf")
                for gi in range(SUPER)
            ]
            for j in range(NJ):
                rhs_t = rhs_pool.tile([P, KC, JW], bf16, tag="rhs")
                nc.sync.dma_start(
                    out=rhs_t[:, :, :],
                    in_=rhsT[:, :, j * JW:(j + 1) * JW].rearrange("kc p j -> p kc j"),
                )
                # MM instructions take at most 2 sync waits; a chunk-leading
                # matmul would need 3 (rhs DMA + psum release + PE). This
                # throwaway matmul is the first consumer of the fresh rhs tile,
                # absorbing the DMA wait into PE's observed clock.
                scr = scr_pool.tile([P, 1], f32, tag="scr", name=f"scr_{s}_{j}")
                nc.tensor.matmul(
                    scr[:, :], lhsT=lhsT_sb[:, 0, 0:P], rhs=rhs_t[:, 0, 0:1],
                    start=True, stop=True,
                )
                for gi in range(SUPER):
                    g = s * SUPER + gi
                    ps = psum_pool.tile([P, JW], f32, tag="ps")
                    for h in range(JW // 512):
                        for kc in range(KC):
                            nc.tensor.matmul(
                                ps[:, h * 512:(h + 1) * 512],
                                lhsT=lhsT_sb[:, kc, g * P:(g + 1) * P],
                                rhs=rhs_t[:, kc, h * 512:(h + 1) * 512],
                                start=(kc == 0),
                                stop=(kc == KC - 1),
                            )
                    nc.scalar.copy(rowbufs[gi][:, j * JW:(j + 1) * JW], ps[:])
            for gi in range(SUPER):
                extract(s * SUPER + gi, rowbufs[gi])

    return nc


def build_program_v2(N=16384, D=512, R=2048, JW=1024, SUPER=4, debug=False,
                     fp8=False):
    """v2e: one DVE full pass per row-group (incremental 128-wide block maxes,
    spread across the column loop so DVE overlaps PE) instead of two.

    Per row there are two argmax candidates: the best across non-self blocks
    (masked block maxes + max_index + per-row indirect gather of the winning
    block from a DRAM copy of the dots), and the best within the self block
    (gathered via a per-core index input, diagonal killed with a constant
    -BIG*I). A branch-free select combines them — necessary because this
    input is locally correlated (~1/3 of true neighbors are in-block).
    """
    KC = D // P
    NG = R // P
    NJ = N // JW
    NS = NG // SUPER
    NBLK = N // P            # 128-wide column blocks per row
    assert D % P == 0 and R % P == 0 and N % JW == 0 and JW % 512 == 0
    assert NG % SUPER == 0 and N % P == 0

    f32 = mybir.dt.float32
    bf16 = mybir.dt.bfloat16
    i32 = mybir.dt.int32
    u32 = mybir.dt.uint32
    Alu = mybir.AluOpType
    NEG = -30000.0
    mmdt = mybir.dt.float8e4 if fp8 else bf16
    assert not fp8 or KC % 2 == 0

    nc = bass.Bass()
    lhsT = nc.declare_dram_parameter("lhsT", [KC, P, R], mmdt, isOutput=False)
    rhsT = nc.declare_dram_parameter("rhsT", [KC, P, N], mmdt, isOutput=False)
    xrows = nc.declare_dram_parameter("xrows", [R, D], f32, isOutput=False)
    xfull = nc.declare_dram_parameter("xfull", [N, D], f32, isOutput=False)
    bmask = nc.declare_dram_parameter("bmask", [P, NG * NBLK], bf16, isOutput=False)
    rbase = nc.declare_dram_parameter("rbase", [P, NG], f32, isOutput=False)
    # self-block support: dots_d row of the self block, its block id, -BIG*I
    rself = nc.declare_dram_parameter("rself", [P, NG], i32, isOutput=False)
    jbself = nc.declare_dram_parameter("jbself", [P, NG], f32, isOutput=False)
    negid = nc.declare_dram_parameter("negid", [P, P], bf16, isOutput=False)
    lnr = nc.declare_dram_parameter("lnr", [R], f32, isOutput=True)
    if debug:
        dbg_idx = nc.declare_dram_parameter("dbg_idx", [R], i32, isOutput=True)
        dbg_jb = nc.declare_dram_parameter("dbg_jb", [R], f32, isOutput=True)
        dbg_off = nc.declare_dram_parameter("dbg_off", [R], f32, isOutput=True)
        dbg_m2 = nc.declare_dram_parameter("dbg_m2", [R], f32, isOutput=True)

    with tile.TileContext(nc) as tc, ExitStack() as ctx:
        const_pool = ctx.enter_context(tc.tile_pool(name="const", bufs=1))
        rhs_pool = ctx.enter_context(tc.tile_pool(name="rhs", bufs=3))
        row_pool = ctx.enter_context(tc.tile_pool(name="row", bufs=SUPER))
        psum_pool = ctx.enter_context(tc.tile_pool(name="ps", bufs=4, space="PSUM"))
        ext_pool = ctx.enter_context(tc.tile_pool(name="ext", bufs=2))
        bmax_pool = ctx.enter_context(tc.tile_pool(name="bmax", bufs=2 * SUPER))
        dram_pool = ctx.enter_context(tc.tile_pool(name="dram", bufs=2 * SUPER,
                                                   space="DRAM"))

        lhsT_sb = const_pool.tile([P, KC, R], mmdt)
        nc.sync.dma_start(
            out=lhsT_sb[:, :, :], in_=lhsT[:, :, :].rearrange("kc p r -> p kc r")
        )
        bmask_sb = const_pool.tile([P, NG * NBLK], bf16)
        nc.sync.dma_start(out=bmask_sb[:, :], in_=bmask[:, :])
        rbase_sb = const_pool.tile([P, NG], f32)
        nc.sync.dma_start(out=rbase_sb[:, :], in_=rbase[:, :])
        rself_sb = const_pool.tile([P, NG], i32)
        nc.sync.dma_start(out=rself_sb[:, :], in_=rself[:, :])
        jbself_sb = const_pool.tile([P, NG], f32)
        nc.sync.dma_start(out=jbself_sb[:, :], in_=jbself[:, :])
        negid_sb = const_pool.tile([P, P], bf16)
        nc.sync.dma_start(out=negid_sb[:, :], in_=negid[:, :])

        def extract(g, b128, dots_d):
            """Small-op argmax finish + gathers; does not touch the rowbuf.

            Two candidates per row: the best across non-self blocks (from the
            masked block maxes), and the best within the self block (the block
            holding the diagonal — gathered and diag-killed, since this input
            has strong local correlation and ~1/3 of true neighbors are
            in-block). A branch-free select combines them.
            """
            # non-self: block maxes with the self block masked out
            b128m = ext_pool.tile([P, NBLK], bf16, tag="b128m")
            nc.vector.tensor_tensor(
                out=b128m[:], in0=b128[:],
                in1=bmask_sb[:, g * NBLK:(g + 1) * NBLK], op=Alu.add,
            )
            m2ns = ext_pool.tile([P, 1], bf16, tag="m2ns")
            nc.vector.tensor_reduce(
                m2ns[:], b128m[:], axis=mybir.AxisListType.X, op=Alu.max
            )
            # self block: gather, kill the diagonal, take its max
            blks = ext_pool.tile([P, P], bf16, tag="blks")
            nc.gpsimd.indirect_dma_start(
                out=blks[:], out_offset=None, in_=dots_d[:, :],
                in_offset=bass.IndirectOffsetOnAxis(ap=rself_sb[:, g:g + 1], axis=0),
            )
            blksk = ext_pool.tile([P, P], bf16, tag="blksk")
            nc.vector.tensor_tensor(
                out=blksk[:], in0=blks[:], in1=negid_sb[:, :], op=Alu.add
            )
            selfmax = ext_pool.tile([P, 1], bf16, tag="selfmax")
            nc.vector.tensor_reduce(
                selfmax[:], blksk[:], axis=mybir.AxisListType.X, op=Alu.max
            )
            # sel = 1.0 where the self block wins
            sel = ext_pool.tile([P, 1], f32, tag="sel")
            nc.vector.tensor_tensor(
                out=sel[:], in0=m2ns[:], in1=selfmax[:], op=Alu.is_lt
            )
            # non-self path: winning block + offset within it
            m8f = ext_pool.tile([P, 8], bf16, tag="m8f")
            nc.vector.memset(m8f[:], NEG)
            nc.vector.tensor_copy(m8f[:, 0:1], m2ns[:])
            jb8 = ext_pool.tile([P, 8], u32, tag="jb8")
            nc.vector.max_index(jb8[:], m8f[:], b128m[:])
            jbns = ext_pool.tile([P, 1], f32, tag="jbns")
            nc.vector.tensor_copy(jbns[:], jb8[:, 0:1])
            rf = ext_pool.tile([P, 1], f32, tag="rf")
            nc.vector.tensor_tensor(
                out=rf[:], in0=jbns[:], in1=rbase_sb[:, g:g + 1], op=Alu.add
            )
            ri = ext_pool.tile([P, 1], i32, tag="ri")
            nc.vector.tensor_copy(ri[:], rf[:])
            blkn = ext_pool.tile([P, P], bf16, tag="blkn")
            nc.gpsimd.indirect_dma_start(
                out=blkn[:], out_offset=None, in_=dots_d[:, :],
                in_offset=bass.IndirectOffsetOnAxis(ap=ri[:, :1], axis=0),
            )
            off8 = ext_pool.tile([P, 8], u32, tag="off8")
            nc.vector.max_index(off8[:], m8f[:], blkn[:])
            offns = ext_pool.tile([P, 1], f32, tag="offns")
            nc.vector.tensor_copy(offns[:], off8[:, 0:1])
            # self path: offset of selfmax within the killed self block
            m8fs = ext_pool.tile([P, 8], bf16, tag="m8fs")
            nc.vector.memset(m8fs[:], NEG)
            nc.vector.tensor_copy(m8fs[:, 0:1], selfmax[:])
            off8s = ext_pool.tile([P, 8], u32, tag="off8s")
            nc.vector.max_index(off8s[:], m8fs[:], blksk[:])
            offs = ext_pool.tile([P, 1], f32, tag="offs")
            nc.vector.tensor_copy(offs[:], off8s[:, 0:1])
            # branch-free select: v = v_ns + sel * (v_self - v_ns)
            jbd = ext_pool.tile([P, 1], f32, tag="jbd")
            nc.vector.tensor_tensor(
                out=jbd[:], in0=jbself_sb[:, g:g + 1], in1=jbns[:], op=Alu.subtract
            )
            jbdm = ext_pool.tile([P, 1], f32, tag="jbdm")
            nc.vector.tensor_tensor(out=jbdm[:], in0=jbd[:], in1=sel[:], op=Alu.mult)
            jbf = ext_pool.tile([P, 1], f32, tag="jbf")
            nc.vector.tensor_tensor(out=jbf[:], in0=jbns[:], in1=jbdm[:], op=Alu.add)
            offd = ext_pool.tile([P, 1], f32, tag="offd")
            nc.vector.tensor_tensor(
                out=offd[:], in0=offs[:], in1=offns[:], op=Alu.subtract
            )
            offdm = ext_pool.tile([P, 1], f32, tag="offdm")
            nc.vector.tensor_tensor(out=offdm[:], in0=offd[:], in1=sel[:], op=Alu.mult)
            offf = ext_pool.tile([P, 1], f32, tag="offf")
            nc.vector.tensor_tensor(out=offf[:], in0=offns[:], in1=offdm[:], op=Alu.add)
            # idx = jb*128 + off
            idxf = ext_pool.tile([P, 1], f32, tag="idxf")
            nc.vector.scalar_tensor_tensor(
                idxf[:], jbf[:], float(P), offf[:], op0=Alu.mult, op1=Alu.add
            )
            idx32 = ext_pool.tile([P, 1], i32, tag="idx32")
            nc.vector.tensor_copy(idx32[:], idxf[:])
            if debug:
                sl = slice(g * P, (g + 1) * P)
                nc.sync.dma_start(out=dbg_idx[sl, None], in_=idx32[:, :1])
                nc.sync.dma_start(out=dbg_jb[sl, None], in_=jbf[:, :1])
                nc.sync.dma_start(out=dbg_off[sl, None], in_=offf[:, :1])
                m2f = ext_pool.tile([P, 1], f32, tag="m2f")
                nc.vector.tensor_copy(m2f[:], m2ns[:])
                nc.sync.dma_start(out=dbg_m2[sl, None], in_=m2f[:, :1])

            gath = ext_pool.tile([P, D], f32, tag="gath")
            nc.gpsimd.indirect_dma_start(
                out=gath[:], out_offset=None, in_=xfull[:, :],
                in_offset=bass.IndirectOffsetOnAxis(ap=idx32[:, :1], axis=0),
            )
            own = ext_pool.tile([P, D], f32, tag="own")
            nc.sync.dma_start(out=own[:], in_=xrows[g * P:(g + 1) * P, :])
            diff = ext_pool.tile([P, D], f32, tag="diff")
            nc.vector.scalar_tensor_tensor(
                diff[:], own[:], 1e-6, gath[:], op0=Alu.add, op1=Alu.subtract
            )
            sq = ext_pool.tile([P, D], f32, tag="sq")
            rho2 = ext_pool.tile([P, 1], f32, tag="rho2")
            nc.vector.scalar_tensor_tensor(
                sq[:], diff[:], 1.0, diff[:], op0=Alu.mult, op1=Alu.mult,
                accum_out=rho2[:],
            )
            lnt = ext_pool.tile([P, 1], f32, tag="lnt")
            nc.scalar.activation(lnt[:], rho2[:], mybir.ActivationFunctionType.Ln)
            nc.sync.dma_start(out=lnr[g * P:(g + 1) * P, None], in_=lnt[:, :1])

        JB = JW // P  # 128-blocks per column chunk
        for s in range(NS):
            rowbufs = [
                row_pool.tile([P, N], bf16, name=f"rowbuf_{s}_{gi}", tag="rowbuf")
                for gi in range(SUPER)
            ]
            b128s = [
                bmax_pool.tile([P, NBLK], bf16, name=f"b128_{s}_{gi}", tag="b128")
                for gi in range(SUPER)
            ]
            dots = [
                dram_pool.tile([P * NBLK, P], bf16, name=f"dots_{s}_{gi}", tag="dots")
                for gi in range(SUPER)
            ]
            for j in range(NJ):
                rhs_t = rhs_pool.tile([P, KC, JW], mmdt, tag="rhs")
                nc.sync.dma_start(
                    out=rhs_t[:, :, :],
                    in_=rhsT[:, :, j * JW:(j + 1) * JW].rearrange("kc p j -> p kc j"),
                )
                for gi in range(SUPER):
                    g = s * SUPER + gi
                    ps = psum_pool.tile([P, JW], f32, tag="ps")
                    for h in range(JW // 512):
                        if fp8:
                            # DoubleRow: K=256 per MM via [K, 2, dim] planes
                            for kc2 in range(KC // 2):
                                nc.tensor.matmul(
                                    ps[:, h * 512:(h + 1) * 512],
                                    lhsT=lhsT_sb[:, 2 * kc2:2 * kc2 + 2,
                                                 g * P:(g + 1) * P],
                                    rhs=rhs_t[:, 2 * kc2:2 * kc2 + 2,
                                              h * 512:(h + 1) * 512],
                                    start=(kc2 == 0),
                                    stop=(kc2 == KC // 2 - 1),
                                    perf_mode=mybir.MatmulPerfMode.DoubleRow,
                                )
                        else:
                            for kc in range(KC):
                                nc.tensor.matmul(
                                    ps[:, h * 512:(h + 1) * 512],
                                    lhsT=lhsT_sb[:, kc, g * P:(g + 1) * P],
                                    rhs=rhs_t[:, kc, h * 512:(h + 1) * 512],
                                    start=(kc == 0),
                                    stop=(kc == KC - 1),
                                )
                    rbslice = rowbufs[gi][:, j * JW:(j + 1) * JW]
                    nc.scalar.copy(rbslice, ps[:])
                    # incremental 128-block maxes: spreads DVE across the j
                    # loop so it overlaps PE instead of bunching at super end
                    nc.vector.tensor_reduce(
                        b128s[gi][:, j * JB:(j + 1) * JB],
                        rbslice.rearrange("p (b o) -> p b o", o=P),
                        axis=mybir.AxisListType.X, op=Alu.max,
                    )
                    # stream this dots slice to DRAM for the later block gather
                    nc.sync.dma_start(
                        out=dots[gi][:, :]
                        .rearrange("(p b) o -> p (b o)", p=P)[:, j * JW:(j + 1) * JW],
                        in_=rbslice,
                    )
            for gi in range(SUPER):
                extract(s * SUPER + gi, b128s[gi], dots[gi])

    return nc


_MULTI_WAIT_OK = ("InstEventSemaphore",)
_DMA_NAMES = ("DMA", "TensorLoad", "TensorSave", "PagedWriteback")


def legalize_waits(nc, max_waits=1):
    """Hardware instruction structs accept a single sync wait; Tile can emit
    more.

    - Engine-stream instructions (matmul, DVE/ACT ops, drains) and
      GPSIMD-issued (SWDGE) DMAs: excess waits move onto same-engine
      EventSemaphore instructions inserted right before the offender —
      engines execute their stream in order, so semantics are identical.
    - HWDGE DMAs are static DGE-queue descriptors, NOT gated by any engine
      stream. Their waits are rerouted through a proxy semaphore: an SP-stream
      EventSemaphore chain consumes each original wait in order, then bumps
      the proxy; the descriptor's single wait slot gets `proxy >= n`.
    """
    n_split = 0
    POOL = mybir.EngineType.Pool
    for func in nc.m.functions:
        # running completion totals per DMA-queue semaphore, in program order
        # (same-queue descriptors execute in program order)
        queue_total = {}
        for bb in func.blocks:
            out = []
            changed = False
            for ins in bb.instructions:
                si = ins.sync_info
                waits = list(si.on_wait) if si and si.on_wait else []
                tn = type(ins).__name__
                is_dma = any(d in tn for d in _DMA_NAMES)
                upd = list(si.on_update) if si and si.on_update else []
                own_names = {
                    str(u.ant_name) for u in upd if "DMA" in str(u.ant_name)
                }
                if len(waits) <= max_waits or tn in _MULTI_WAIT_OK:
                    out.append(ins)
                    for u in upd:
                        n = str(u.ant_name)
                        if "DMA" in n:
                            queue_total[n] = queue_total.get(n, 0) + u.update_value
                    continue
                if is_dma:
                    # Drop redundant waits on the DMA's OWN queue semaphore:
                    # descriptors on a DGE queue execute in order, so a wait
                    # for this queue's earlier completions is implied. (The
                    # Tile pass that removes these, optimize_sems, is
                    # disabled.)
                    def implied(w):
                        n = str(w.ant_name)
                        return n in own_names and w.wait_value <= queue_total.get(n, 0)

                    waits = [w for w in waits if not implied(w)]
                    changed = True
                    if len(waits) <= max_waits:
                        ins.sync_info = mybir.SyncInfo(on_wait=waits, on_update=upd)
                        out.append(ins)
                        for u in upd:
                            n = str(u.ant_name)
                            if "DMA" in n:
                                queue_total[n] = queue_total.get(n, 0) + u.update_value
                        continue
                    if ins.engine != POOL:
                        # Rare fallback: reroute through a proxy semaphore.
                        # EVSEMs consume the original waits on an engine
                        # stream (PE self-waits drain; cross-queue input
                        # loads don't depend on future engine work), then
                        # bump the proxy; the descriptor waits on it.
                        eng = None
                        for w in waits:
                            n = str(w.ant_name)
                            for cand, e in (("PE", mybir.EngineType.PE),
                                            ("Activation", mybir.EngineType.Activation),
                                            ("DVE", mybir.EngineType.DVE)):
                                if n.startswith(cand):
                                    eng = e
                                    break
                            if eng is not None:
                                break
                        if eng is None:
                            eng = mybir.EngineType.SP
                        h = nc.alloc_semaphore(f"wproxy_{ins.name}")
                        for k, w in enumerate(waits):
                            ev_upd = []
                            if k == len(waits) - 1:
                                ev_upd = [mybir.SyncUpdate(
                                    sync_type="semaphore", id=h.num,
                                    ant_name=h.name, update_mode="sem-inc",
                                    update_value=1, update_reg=None)]
                            out.append(mybir.InstEventSemaphore(
                                name=f"{ins.name}-wproxy-{k}", engine=eng,
                                ins=[], outs=[],
                                sync_info=mybir.SyncInfo(on_wait=[w],
                                                         on_update=ev_upd)))
                        ins.sync_info = mybir.SyncInfo(
                            on_wait=[mybir.SyncWait(
                                sync_type="semaphore", id=h.num,
                                ant_name=h.name, wait_mode="sem-ge-imm",
                                wait_value=1, wait_reg=None)],
                            on_update=upd)
                        out.append(ins)
                        for u in upd:
                            n = str(u.ant_name)
                            if "DMA" in n:
                                queue_total[n] = queue_total.get(n, 0) + u.update_value
                        n_split += 1
                        continue
                    # SWDGE (Pool-issued): stream-gated, EVSEM split is valid
                changed = True
                n_split += 1
                excess, keep = waits[:-max_waits], waits[-max_waits:]
                for k, w in enumerate(excess):
                    out.append(mybir.InstEventSemaphore(
                        name=f"{ins.name}-wsplit-{k}",
                        engine=ins.engine, ins=[], outs=[],
                        sync_info=mybir.SyncInfo(on_wait=[w], on_update=[]),
                    ))
                ins.sync_info = mybir.SyncInfo(on_wait=keep, on_update=upd)
                out.append(ins)
                for u in upd:
                    n = str(u.ant_name)
                    if "DMA" in n:
                        queue_total[n] = queue_total.get(n, 0) + u.update_value
            if changed:
                try:
                    bb.instructions[:] = out
                except TypeError:
                    bb.instructions = out
    return n_split


def make_in_maps_v2(x, ncores=8, fp8=False):
    N, D = x.shape
    R = N // ncores
    KC = D // P
    NG = R // P
    NBLK = N // P
    xb = x.astype(ml_dtypes.float8_e4m3 if fp8 else ml_dtypes.bfloat16)
    xT = np.ascontiguousarray(xb.T).reshape(KC, P, N)
    # row index into the per-group dots tile: p*NBLK + jb (g-independent)
    rbase = np.empty((P, NG), dtype=np.float32)
    for g in range(NG):
        rbase[:, g] = np.arange(P) * NBLK
    negid = np.zeros((P, P), dtype=ml_dtypes.bfloat16)
    np.fill_diagonal(negid, -30000.0)
    in_maps = []
    for c in range(ncores):
        bm = np.zeros((P, NG * NBLK), dtype=ml_dtypes.bfloat16)
        rs = np.empty((P, NG), dtype=np.int32)
        jbs = np.empty((P, NG), dtype=np.float32)
        for g in range(NG):
            bm[:, g * NBLK + (c * NG + g)] = -30000.0
            rs[:, g] = np.arange(P) * NBLK + c * NG + g
            jbs[:, g] = c * NG + g
        in_maps.append({
            "lhsT": np.ascontiguousarray(xT[:, :, c * R:(c + 1) * R]),
            "rhsT": xT,
            "xrows": np.ascontiguousarray(x[c * R:(c + 1) * R]),
            "xfull": x,
            "bmask": bm,
            "rbase": rbase,
            "rself": rs,
            "jbself": jbs,
            "negid": negid,
        })
    return in_maps


def make_in_maps(x, ncores=8):
    """Shard/transform the full input for each core."""
    N, D = x.shape
    R = N // ncores
    KC = D // P
    xb = x.astype(ml_dtypes.bfloat16)
    # xT[kc, k, j] = x[j, kc*128 + k]
    xT = np.ascontiguousarray(xb.T).reshape(KC, P, N)
    in_maps = []
    for c in range(ncores):
        in_maps.append({
            "lhsT": np.ascontiguousarray(xT[:, :, c * R:(c + 1) * R]),
            "rhsT": xT,
            "xrows": np.ascontiguousarray(x[c * R:(c + 1) * R]),
            "xfull": x,
        })
    return in_maps


_CACHED = {}


# fp8 DoubleRow (1.33x model speedup, accuracy-safe per host sim) passes
# CoreSim exactly but returns NaN on silicon even without any non-DoubleRow
# matmuls in the stream — the DoubleRow weight layout the simulator accepts
# ([K, 2, M] planes) evidently differs from what the hardware weight path
# expects. bf16 is the validated configuration (~467 us model time).
_FP8 = False


def _get_program():
    if "nc" not in _CACHED:
        nc = build_program_v2(fp8=_FP8)
        legalize_waits(nc)
        _CACHED["nc"] = nc
    return _CACHED["nc"]


def kernel(x: np.ndarray) -> np.ndarray:
    x = np.ascontiguousarray(np.asarray(x, dtype=np.float32))
    ncores = 8
    nc = _get_program()
    in_maps = make_in_maps_v2(x, ncores, fp8=_FP8)
    res = run_bass_kernel_spmd(nc, in_maps, list(range(ncores)))
    lnr = np.concatenate([np.asarray(r["lnr"]) for r in res.results])
    loss = -0.1 * 0.5 * float(np.mean(lnr.astype(np.float64)))
    return np.float32(loss)

